# revision 37
# baseline (speedup 1.0000x reference)
"""Trainium2 Bass kernel for nn_DenoisingTransformer (linear attention block).

Computation (see reference):
  q,k,v = x@Wq, x@Wk, x@Wv  (16 heads of 64)
  q,k = relu(rope(q)), relu(rope(k))      (interleaved-pair rope)
  vk[b,h,e,d] = sum_s v_pad[b,h,s,e] * k[b,h,s,d]   (v padded with ones col)
  num = q . vk ; attn = num[:,:64] / (num[:,64] + eps) ; out = attn @ Wo

Sharding: 8 cores = (batch 4) x (head-halves 2). Each core processes the FULL
4096-token sequence for its 8 heads, producing a partial output
y_part = attn_half @ Wo[rows of its heads]; the host sums the two partials
per batch. No device collective is needed (vk is per-head local).

Layout tricks (all host-side prep, exact):
 - x is pre-transposed+tiled+cast to bf16 on the host: xt[p, t*1024+c*128+n]
   = x[b, t*128+n, c*128+p], so the d_model contraction dim is on partitions
   with contiguous 2KB-per-partition DMA. No on-chip transposes of x.
 - Wq/Wk columns are de-interleaved per head (evens|odds) so rope operates on
   contiguous 32-col blocks; consistent through q.k contractions, so exact.
 - Weights/cos/sin pre-arranged so every DMA is contiguous per partition.
 - q and attn transposes run on the DMA engines (XBAR dma_start_transpose),
   keeping the PE free for matmuls.
 - vk accumulates across all 32 tiles directly in PSUM (one bank per head
   pair, one long accumulation group), not via DVE adds.
"""

import numpy as np

import concourse.bacc as bacc
import concourse.mybir as mybir
import concourse.tile as tile
from concourse.masks import make_identity

F32 = mybir.dt.float32
BF16 = mybir.dt.bfloat16

D = 1024
H_LOC = 8          # heads per core
HD = 64
NPAIR = 4          # head pairs per core
THETA = 10000.0
EPS = 1e-6

B_FULL, S_FULL = 4, 4096
N_CORES = 8
S_LOC = S_FULL     # full sequence per core
VKW = 129          # vk psum width per pair (128 v cols + 1 ksum)
NUMW = 65          # per-head num width (64 + den)


def build_program(s_loc=S_LOC, n_cores=N_CORES, dtype_mode="bf16"):
    T = s_loc // 128
    WDT = BF16

    nc = bacc.Bacc("TRN2", target_bir_lowering=False, num_devices=n_cores)

    xt_d = nc.dram_tensor("xt", [128, T * D], WDT, kind="ExternalInput")
    wq_d = nc.dram_tensor("wq", [128, 8 * 512], WDT, kind="ExternalInput")
    wk_d = nc.dram_tensor("wk", [128, 8 * 512], WDT, kind="ExternalInput")
    wv_d = nc.dram_tensor("wv", [128, 8 * 512], WDT, kind="ExternalInput")
    wo_d = nc.dram_tensor("wo", [128, 4 * D], WDT, kind="ExternalInput")
    cos_d = nc.dram_tensor("cos_t", [128, T * 32], F32, kind="ExternalInput")
    sin_d = nc.dram_tensor("sin_t", [128, T * 32], F32, kind="ExternalInput")
    y_d = nc.dram_tensor("y", [s_loc, D], F32, kind="ExternalOutput")

    def mm(dst, lhsT, rhs, start, stop):
        nc.tensor.matmul(dst, lhsT=lhsT, rhs=rhs, start=start, stop=stop)

    with tile.TileContext(nc) as tc:
        with (
            tc.tile_pool(name="const", bufs=1) as constp,
            tc.tile_pool(name="wpool", bufs=1) as wpool,
            tc.tile_pool(name="xall", bufs=1) as xallp,
            tc.tile_pool(name="work", bufs=3) as wk,
            tc.tile_pool(name="io", bufs=3) as iop,
        ):
            # identity first: it is engine-generated (no DMA), so the warmup
            # matmuls below can start while weights are still in flight
            ident = constp.tile([128, 128], F32, tag="idf")
            make_identity(nc, ident[:])
            ident_s = constp.tile([128, 128], WDT, tag="idb")
            nc.vector.tensor_copy(ident_s[:], ident[:])

            # ---- resident inputs ----
            # Startup is HBM-bound: prioritize what phase-1 tile 0 needs.
            # xt tiles are throttled (6-tile lookahead, issued in the loop);
            # wk/wv chunks split across the gpsimd+scalar rings.
            xT_all = xallp.tile([128, T * D], WDT, tag="xT")

            def xt_load_pair(p):
                # paired tiles: 4KB-per-partition descriptors halve DMA
                # descriptor-processing overhead
                nc.sync.dma_start(
                    xT_all[:, 2 * p * D : (2 * p + 2) * D],
                    xt_d[:, 2 * p * D : (2 * p + 2) * D],
                )

            # whole-tensor weight DMAs: 8KB contiguous per partition minimizes
            # descriptor count (descriptor processing, not HBM bandwidth, is
            # the startup bottleneck)
            # Each DMA ring sustains only ~90GB/s, so startup transfers are
            # balanced across the three rings in need-time order:
            #   sync:   xt pair0, cosA, sinA, pairs 1-3  (then in-loop pairs)
            #   gpsimd: wkA, wvA, wq, wo
            #   scalar: wkB, wvB, cosB, sinB
            wk_sb = wpool.tile([128, 8 * 512], WDT, tag="wa")
            wv_sb = wpool.tile([128, 8 * 512], WDT, tag="wb")
            wq_sb = wpool.tile([128, 8 * 512], WDT, tag="wc")
            wo_sb = wpool.tile([128, 4 * D], WDT, tag="wd")
            cos_all = constp.tile([128, T * 32], F32, tag="cos")
            sin_all = constp.tile([128, T * 32], F32, tag="sin")
            wmid = 4 * 512
            half = T * 16
            nc.gpsimd.dma_start(wk_sb[:, 0:wmid], wk_d[:, 0:wmid])
            nc.scalar.dma_start(wk_sb[:, wmid:], wk_d[:, wmid:])
            xt_load_pair(0)
            nc.gpsimd.dma_start(wv_sb[:, 0:wmid], wv_d[:, 0:wmid])
            nc.scalar.dma_start(wv_sb[:, wmid:], wv_d[:, wmid:])
            nc.sync.dma_start(cos_all[:, 0:half], cos_d[:, 0:half])
            nc.sync.dma_start(sin_all[:, 0:half], sin_d[:, 0:half])
            xt_load_pair(1)
            xt_load_pair(2)
            xt_load_pair(3)
            nc.gpsimd.dma_start(wq_sb[:], wq_d[:])
            nc.gpsimd.dma_start(wo_sb[:], wo_d[:])
            nc.scalar.dma_start(cos_all[:, half:], cos_d[:, half:])
            nc.scalar.dma_start(sin_all[:, half:], sin_d[:, half:])

            vkT_sb = constp.tile([128, 2 * NPAIR * NUMW], WDT, tag="vkT")
            nc.vector.memset(vkT_sb[:], 0.0)

            def rope(psrc, t, dst, eng):
                """rope 8 heads: psrc [128, 512] (per head: 32 evens | 32 odds)."""
                csb = cos_all[:, t * 32 : (t + 1) * 32]
                ssb = sin_all[:, t * 32 : (t + 1) * 32]
                e3 = psrc[:].rearrange("p (h d) -> p h d", h=8)[:, :, 0:32]
                o3 = psrc[:].rearrange("p (h d) -> p h d", h=8)[:, :, 32:64]
                cb = csb.unsqueeze(1).broadcast_to([128, 8, 32])
                sb_ = ssb.unsqueeze(1).broadcast_to([128, 8, 32])
                t1 = wk.tile([128, 256], F32, tag="rt1")
                t2 = wk.tile([128, 256], F32, tag="rt2")
                t13 = t1[:].rearrange("p (h d) -> p h d", h=8)
                t23 = t2[:].rearrange("p (h d) -> p h d", h=8)
                d3 = dst[:].rearrange("p (h d) -> p h d", h=8)
                eng.tensor_mul(t13, e3, cb)
                eng.tensor_mul(t23, o3, sb_)
                eng.tensor_sub(d3[:, :, 0:32], t13, t23)
                eng.tensor_mul(t13, e3, sb_)
                eng.tensor_mul(t23, o3, cb)
                eng.tensor_add(d3[:, :, 32:64], t13, t23)

            # ---------------- phase 1: k, v, vk (PSUM-accumulated) ----------
            with (
                tc.tile_pool(name="psP", bufs=4, space="PSUM") as psP,
                tc.tile_pool(name="psVK", bufs=4, space="PSUM") as psVK,
            ):
                # dummy matmuls on the identity ramp the PE clock (HAM) out of
                # its idle throttle while the first weights are still in
                # flight, so the real matmuls start at full speed
                pwarm = psP.tile([128, 512], F32, tag="pp", name="pwarm")
                rhs_w = ident_s[:].unsqueeze(1).broadcast_to([128, 4, 128])
                for i in range(64):
                    nc.tensor.matmul(
                        pwarm[:].rearrange("p (q c) -> p q c", q=4), lhsT=ident_s[:],
                        rhs=rhs_w, start=True, stop=True,
                    )

                vkps = [
                    psVK.tile([128, 512], F32, tag="vk", name=f"vkps{p}")
                    for p in range(NPAIR)
                ]
                krs = {}
                vss = {}

                def kfin(t, pk):
                    kr_sb = wk.tile([128, 512], WDT, tag="kr", name=f"kr{t}")
                    rope(pk, t, kr_sb, nc.vector)
                    nc.scalar.activation(
                        kr_sb[:], kr_sb[:], mybir.ActivationFunctionType.Relu
                    )
                    krs[t] = kr_sb

                def vfin(t, pv):
                    v_sb = wk.tile([128, NPAIR * VKW], WDT, tag="v", name=f"v{t}")
                    nc.scalar.copy(
                        v_sb[:].rearrange("p (q c) -> p q c", q=NPAIR)[:, :, 0:128],
                        pv[:].rearrange("p (q c) -> p q c", q=NPAIR),
                    )
                    nc.gpsimd.memset(
                        v_sb[:].rearrange("p (q c) -> p q c", q=NPAIR)[:, :, 128:129],
                        1.0,
                    )
                    vss[t] = v_sb

                def proj(pdst, t, w_sb, c):
                    mm(
                        pdst[:],
                        xT_all[:, t * D + c * 128 : t * D + (c + 1) * 128],
                        w_sb[:, c * 512 : (c + 1) * 512],
                        start=(c == 0),
                        stop=(c == 7),
                    )

                def kvchain(t):
                    nxt = t + 4
                    if nxt % 2 == 0 and nxt < T:
                        xt_load_pair(nxt // 2)
                    pk = psP.tile([128, 512], F32, tag="pp", name=f"pk{t}")
                    for c in range(8):
                        proj(pk, t, wk_sb, c)
                    kfin(t, pk)
                    pv = psP.tile([128, 512], F32, tag="pp", name=f"pv{t}")
                    for c in range(8):
                        proj(pv, t, wv_sb, c)
                    vfin(t, pv)

                def warmup_kv(tiles):
                    # chunk-major over the first tiles: each arriving 128KB
                    # weight slice feeds len(tiles) matmuls, not one
                    pks = {t: psP.tile([128, 512], F32, tag="pp", name=f"pk{t}") for t in tiles}
                    for c in range(8):
                        for t in tiles:
                            proj(pks[t], t, wk_sb, c)
                    for t in tiles:
                        kfin(t, pks[t])
                    pvs = {t: psP.tile([128, 512], F32, tag="pp", name=f"pv{t}") for t in tiles}
                    for c in range(8):
                        for t in tiles:
                            proj(pvs[t], t, wv_sb, c)
                    for t in tiles:
                        vfin(t, pvs[t])

                def vkstep(t):
                    kr_sb, v_sb = krs.pop(t), vss.pop(t)
                    for p in range(NPAIR):
                        mm(
                            vkps[p][:, 0:VKW],
                            kr_sb[:, p * 128 : (p + 1) * 128],
                            v_sb[:, p * VKW : (p + 1) * VKW],
                            start=(t == 0),
                            stop=(t == T - 1),
                        )

                warmup_kv([0, 1, 2])
                for t in range(T):
                    if t + 3 < T:
                        kvchain(t + 3)
                    vkstep(t)

                # reorganize vk psum -> vkT_sb (bf16, zero cross-blocks).
                # On scalar: vector must go straight into rope(q0) here.
                for p in range(NPAIR):
                    ps = vkps[p]
                    nc.scalar.copy(
                        vkT_sb[0:64, p * 2 * NUMW : p * 2 * NUMW + 64],
                        ps[0:64, 0:64],
                    )
                    nc.scalar.copy(
                        vkT_sb[0:64, p * 2 * NUMW + 64 : p * 2 * NUMW + 65],
                        ps[0:64, 128:129],
                    )
                    nc.scalar.copy(
                        vkT_sb[64:128, p * 2 * NUMW + 65 : p * 2 * NUMW + 130],
                        ps[64:128, 64:129],
                    )

            # ---------------- phase 2: q, num, attn, out (pipelined) --------
            with (
                tc.tile_pool(name="psQ", bufs=2, space="PSUM") as psQ,
                tc.tile_pool(name="psT", bufs=2, space="PSUM") as psT,
                tc.tile_pool(name="psN", bufs=2, space="PSUM") as psN,
                tc.tile_pool(name="psO", bufs=2, space="PSUM") as psO,
            ):
                qrs = {}
                qts = {}
                attns = {}
                attnTs = {}

                def qproj_rope(t):
                    xt_t = xT_all[:, t * D : (t + 1) * D]
                    pq = psQ.tile([128, 512], F32, tag="pq", name=f"pq{t}")
                    for c in range(8):
                        mm(
                            pq[:],
                            xt_t[:, c * 128 : (c + 1) * 128],
                            wq_sb[:, c * 512 : (c + 1) * 512],
                            start=(c == 0),
                            stop=(c == 7),
                        )
                    qr_sb = wk.tile([128, 512], WDT, tag="qr", name=f"qr{t}")
                    rope(pq, t, qr_sb, nc.vector)
                    qrs[t] = qr_sb

                def qtrans(t):
                    qr_sb = qrs.pop(t)
                    pqt = psT.tile([128, 512], WDT, tag="tp", name=f"pqt{t}")
                    for p in range(NPAIR):
                        nc.tensor.transpose(
                            pqt[:, p * 128 : (p + 1) * 128],
                            qr_sb[:, p * 128 : (p + 1) * 128],
                            ident_s[:],
                        )
                    qT_sb = wk.tile([128, 512], WDT, tag="qT", name=f"qT{t}")
                    nc.scalar.activation(
                        qT_sb[:], pqt[:], mybir.ActivationFunctionType.Relu
                    )
                    qts[t] = qT_sb

                def numstep(t):
                    qT_sb = qts.pop(t)
                    pns = []
                    for bi in range(2):
                        pn = psN.tile([128, 4 * NUMW], F32, tag="num", name=f"pn{t}_{bi}")
                        pns.append(pn)
                        for pp in range(2):
                            p = bi * 2 + pp
                            mm(
                                pn[:, pp * 2 * NUMW : (pp + 1) * 2 * NUMW],
                                qT_sb[:, p * 128 : (p + 1) * 128],
                                vkT_sb[:, p * 2 * NUMW : (p + 1) * 2 * NUMW],
                                start=True,
                                stop=True,
                            )
                    den = wk.tile([128, 2 * NPAIR], F32, tag="den", name=f"den{t}")
                    for bi in range(2):
                        nc.vector.tensor_scalar_add(
                            den[:, 4 * bi : 4 * bi + 4], pns[bi][:, 64::NUMW], EPS
                        )
                    rec = wk.tile([128, 2 * NPAIR], F32, tag="rec", name=f"rec{t}")
                    nc.vector.reciprocal(rec[:], den[:])
                    attn_sb = wk.tile([128, 512], WDT, tag="attn", name=f"attn{t}")
                    for bi in range(2):
                        nc.vector.tensor_mul(
                            attn_sb[:, bi * 256 : (bi + 1) * 256].rearrange(
                                "p (h e) -> p h e", e=64
                            ),
                            pns[bi][:].rearrange("p (h e) -> p h e", e=NUMW)[
                                :, :, 0:64
                            ],
                            rec[:, 4 * bi : 4 * bi + 4]
                            .unsqueeze(2)
                            .broadcast_to([128, 4, 64]),
                        )
                    attns[t] = attn_sb

                def atrans(t):
                    attn_sb = attns.pop(t)
                    pat = psT.tile([128, 512], WDT, tag="tp", name=f"pat{t}")
                    for p in range(NPAIR):
                        nc.tensor.transpose(
                            pat[:, p * 128 : (p + 1) * 128],
                            attn_sb[:, p * 128 : (p + 1) * 128],
                            ident_s[:],
                        )
                    attnT_sb = wk.tile([128, 512], WDT, tag="attnT", name=f"aT{t}")
                    nc.scalar.copy(attnT_sb[:], pat[:])
                    attnTs[t] = attnT_sb

                def ostep(t):
                    attnT_sb = attnTs.pop(t)
                    out_sb = iop.tile([128, D], F32, tag="out", name=f"out{t}")
                    for nb in range(2):
                        po = psO.tile([128, 512], F32, tag="po", name=f"po{t}_{nb}")
                        for c in range(4):
                            mm(
                                po[:],
                                attnT_sb[:, c * 128 : (c + 1) * 128],
                                wo_sb[:, c * D + nb * 512 : c * D + (nb + 1) * 512],
                                start=(c == 0),
                                stop=(c == 3),
                            )
                        if nb == 0:
                            nc.scalar.copy(out_sb[:, 0:512], po[:])
                        else:
                            nc.vector.tensor_copy(out_sb[:, 512:1024], po[:])
                    yeng = (nc.gpsimd, nc.sync, nc.scalar)[t % 3]
                    yeng.dma_start(y_d[t * 128 : (t + 1) * 128, :], out_sb[:])

                qproj_rope(0)
                qproj_rope(1)
                qtrans(0)
                for j in range(T + 2):
                    if j + 2 < T:
                        qproj_rope(j + 2)
                    if j + 1 < T:
                        qtrans(j + 1)
                    if j < T:
                        numstep(j)
                    if j >= 1 and j - 1 < T:
                        atrans(j - 1)
                    if j >= 2:
                        ostep(j - 2)

    nc.compile()
    return nc


# ---------------------------------------------------------------------------
# host side
# ---------------------------------------------------------------------------


def _head_perm():
    """De-interleave permutation for Wq/Wk columns (per head: evens then odds)."""
    perm = np.zeros(D, dtype=np.int64)
    for h in range(16):
        for j in range(32):
            perm[h * HD + j] = h * HD + 2 * j
            perm[h * HD + 32 + j] = h * HD + 2 * j + 1
    return perm


def _rope_tables(s_total):
    freqs = 1.0 / (THETA ** (np.arange(0, HD, 2, dtype=np.float64) / HD))
    ang = np.arange(s_total, dtype=np.float64)[:, None] * freqs[None, :]
    return np.cos(ang).astype(np.float32), np.sin(ang).astype(np.float32)


def _tile_rows(a, T):
    """[T*128, W] -> [128, T*W] with [p, t*W+j] = a[t*128+p, j]."""
    w = a.shape[1]
    return np.ascontiguousarray(
        a.reshape(T, 128, w).transpose(1, 0, 2).reshape(128, T * w)
    )


def make_in_maps(x, Wq, Wk, Wv, Wo, n_cores=N_CORES, dtype_mode="bf16"):
    import ml_dtypes

    wdt = ml_dtypes.bfloat16
    b, s, d = x.shape
    s_loc = s
    T = s_loc // 128
    perm = _head_perm()
    wq_p = np.ascontiguousarray(Wq[:, perm])
    wk_p = np.ascontiguousarray(Wk[:, perm])

    # xt per batch: [p, t*1024 + c*128 + n] = x[b, t*128+n, c*128+p]
    xts = []
    for bi in range(b):
        xr = x[bi].reshape(T, 128, 8, 128).transpose(3, 0, 2, 1)
        xts.append(np.ascontiguousarray(xr.reshape(128, T * d)).astype(wdt))

    def wslice(W, half):
        """[1024, 512] col-slice -> [128, 8*512] with [p, c*512+n] = W[c*128+p, n]."""
        ws = W[:, half * 512 : (half + 1) * 512]
        return np.ascontiguousarray(
            ws.reshape(8, 128, 512).transpose(1, 0, 2).reshape(128, 8 * 512)
        ).astype(wdt)

    def woslice(half):
        """Wo row-slice [512, 1024] -> [128, 4*1024]."""
        ws = Wo[half * 512 : (half + 1) * 512, :]
        return np.ascontiguousarray(
            ws.reshape(4, 128, d).transpose(1, 0, 2).reshape(128, 4 * d)
        ).astype(wdt)

    cos_full, sin_full = _rope_tables(s)
    cos_t = _tile_rows(cos_full, T)
    sin_t = _tile_rows(sin_full, T)

    whs = {
        (nm, half): f(half)
        for half in range(2)
        for nm, f in (
            ("wq", lambda hh: wslice(wq_p, hh)),
            ("wk", lambda hh: wslice(wk_p, hh)),
            ("wv", lambda hh: wslice(Wv, hh)),
            ("wo", woslice),
        )
    }

    in_maps = []
    for c in range(n_cores):
        bi, half = c // 2, c % 2
        in_maps.append(
            {
                "xt": xts[bi],
                "wq": whs[("wq", half)],
                "wk": whs[("wk", half)],
                "wv": whs[("wv", half)],
                "wo": whs[("wo", half)],
                "cos_t": cos_t,
                "sin_t": sin_t,
            }
        )
    return in_maps, s_loc


def assemble_output(x_shape, results):
    b, s, d = x_shape
    out = np.empty((b, s, d), dtype=np.float32)
    for bi in range(b):
        out[bi] = results[2 * bi]["y"] + results[2 * bi + 1]["y"]
    return out


_CACHED = {}


def kernel(x, Wq, Wk, Wv, Wo):
    from concourse.bass_utils import run_bass_kernel_spmd

    x = np.asarray(x, dtype=np.float32)
    in_maps, s_loc = make_in_maps(
        x,
        np.asarray(Wq, np.float32),
        np.asarray(Wk, np.float32),
        np.asarray(Wv, np.float32),
        np.asarray(Wo, np.float32),
    )
    key = (s_loc, N_CORES)
    if key not in _CACHED:
        _CACHED[key] = build_program(s_loc=s_loc, n_cores=N_CORES)
    nc = _CACHED[key]
    res = run_bass_kernel_spmd(nc, in_maps, list(range(N_CORES)))
    return assemble_output(x.shape, res.results)
